# revision 1
# baseline (speedup 1.0000x reference)
"""Distributed Trainium2 kernel for GNN message passing (COO SpMM + dense head).

out = relu((A @ x) @ W[:128] + x @ W[128:])   with A given as COO (rows, cols, vals)

Strategy (8 NeuronCores, SPMD single graph):
  - Rows (destinations) sharded across cores: core c owns rows [c*12500, (c+1)*12500).
  - x is replicated to every core's DRAM via its input map (no collectives needed).
  - Host-side index preprocessing only (sorting / padding / layout): per core the
    edge list is sorted by col-chunk (4 chunks of 25000 so gather indices fit in
    int16), padded to shared per-chunk capacities so all 8 cores run the same graph.
  - On device per tile of edges: hardware gather x[col] (SWDGE dma_gather),
    scale by vals (VectorE broadcast multiply), hardware scatter-add into a DRAM
    h accumulator (SWDGE dma_scatter_add).  Paired edge occurrences are laid out
    so one 1KB scatter descriptor writes both parity stripes of h[row] at once,
    cutting Q7 descriptor-generation (the measured bottleneck, ~7ns/descriptor)
    from 153K to ~127K descriptors per core.
  - Dense head (batched 4 groups per DMA): combine h stripes, h.T via PE
    transpose, out = relu(hT.T@W1 + xT.T@W2) accumulated in PSUM, relu on
    ScalarE; the xlocT arena is preloaded to SBUF during the SpMM phase.
"""

import sys

if "/opt/trn_rl_repo" not in sys.path:
    sys.path.insert(0, "/opt/trn_rl_repo")

import numpy as np

N_NODES = 100000
N_EDGES = 600000
D = 128
OUT = 128
P = 128
NCORES = 8
RPC = N_NODES // NCORES          # 12500 rows per core
NCHUNK = 4
CHUNK = N_NODES // NCHUNK        # 25000 (< 32768 so int16 gather idx works)
TILE_E = 1024                    # max edges per SWDGE call (1024-descriptor ring limit)
H_PAD = 12800                    # padded row count per parity stripe
MAXROUNDS = 16                   # upper bound on per-(chunk,row) multiplicity / 2

_compiled = {}


def _prep(adj_rows, adj_cols, adj_vals):
    """Per-core uniform-shape gather/scatter metadata (int/layout work only).

    Round j of a (chunk,row) group holds edge occurrences {2j, 2j+1}.  Rows
    with both occurrences present ("paired") are laid out so occurrence 2j
    sits at position b*256+m and 2j+1 at b*256+128+m (same SBUF partition,
    adjacent 128-slots) - one 1KB scatter descriptor then writes both parity
    stripes of h2[row] at once (elem_size=256).  Rows with only occurrence 2j
    ("unpaired") go in separate 512B-token calls into the [25600,128] view.
    Within any call destinations are unique, calls are WAW-serialized, so the
    non-atomic HW scatter-add never races.
    """
    rows = np.asarray(adj_rows).astype(np.int64)
    cols = np.asarray(adj_cols).astype(np.int64)
    vals = np.asarray(adj_vals).astype(np.float32)

    per_core = []
    # sizes[c, chunk, round, type]: type 0 = paired (edge count), 1 = unpaired
    sizes_all = np.zeros((NCORES, NCHUNK, MAXROUNDS, 2), np.int64)
    for c in range(NCORES):
        m = (rows >= c * RPC) & (rows < (c + 1) * RPC)
        r = rows[m] - c * RPC
        co = cols[m]
        v = vals[m]
        ch = co // CHUNK
        o = np.lexsort((r, ch))
        r, co, v, ch = r[o], co[o], v[o], ch[o]
        key = ch * RPC + r
        n = len(key)
        change = np.empty(n, bool)
        if n:
            change[0] = True
            change[1:] = key[1:] != key[:-1]
        starts = np.flatnonzero(change)
        lens = np.diff(np.append(starts, n))
        occ = np.arange(n) - np.repeat(starts, lens)
        kcnt = np.repeat(lens, lens)          # per-edge group size
        rnd = occ // 2
        assert rnd.max(initial=0) < MAXROUNDS
        parity = occ % 2
        paired = (2 * rnd + 1) < kcnt         # partner exists
        typ = (~paired).astype(np.int64)      # 0 = paired, 1 = unpaired
        # a row has exactly one unpaired edge per chunk (its last odd
        # occurrence), so all unpaired tokens of a chunk are distinct:
        # merge them into a single round-0 group -> fewer, fuller calls
        rnd = np.where(typ == 1, 0, rnd)
        # order: (chunk, round, type, parity, row) -> within a paired group the
        # parity-0 run and parity-1 run list the same rows in the same order
        o2 = np.lexsort((r, parity, typ, rnd, ch))
        r, co, v, ch, rnd, parity, typ = (
            a[o2] for a in (r, co, v, ch, rnd, parity, typ))
        for k in range(NCHUNK):
            for t in range(2):
                mk = (ch == k) & (typ == t)
                sizes_all[c, k, :, t] += np.bincount(
                    rnd[mk], minlength=MAXROUNDS)
        per_core.append((r, co % CHUNK, v, ch, rnd, parity, typ))

    caps = np.zeros((NCHUNK, MAXROUNDS, 2), np.int64)
    caps[:, :, 0] = ((sizes_all[:, :, :, 0].max(axis=0) + 255) // 256) * 256
    caps[:, :, 1] = ((sizes_all[:, :, :, 1].max(axis=0) + 127) // 128) * 128
    T = int(caps.sum())

    # static call list: (chunk, dram_offset, n_edges, type)
    calls = []
    off = 0
    for k in range(NCHUNK):
        for j in range(MAXROUNDS):
            for t in range(2):
                cap = int(caps[k, j, t])
                tile_sz = 2048 if t == 0 else TILE_E
                for t0 in range(0, cap, tile_sz):
                    calls.append((k, off + t0, min(tile_sz, cap - t0), t))
                off += cap

    gidx_w = np.zeros((NCORES, P, T // 16), np.int16)
    sp_w = np.zeros((NCORES, P, T // 2 // 16), np.int16)   # paired: idx per pair
    su_w = np.zeros((NCORES, P, T // 16), np.int16)        # unpaired idx
    sval_w = np.zeros((NCORES, P, T // 128), np.float32)
    offs = np.concatenate([[0], np.cumsum(caps.reshape(-1))]).astype(np.int64)

    for c in range(NCORES):
        r, coi, v, ch, rnd, parity, typ = per_core[c]
        gi = np.zeros(T, np.int16)
        sp = np.full(T // 2, 12600, np.int16)   # paired dump row (1KB units)
        su = np.full(T, 25200, np.int16)        # unpaired dump (512B units)
        sv = np.zeros(T, np.float32)
        s = 0
        gidx = 0
        for k in range(NCHUNK):
            for j in range(MAXROUNDS):
                for t in range(2):
                    d0 = int(offs[gidx]); gidx += 1
                    nn = int(sizes_all[c, k, j, t])
                    if t == 1:
                        gi[d0:d0 + nn] = coi[s:s + nn]
                        su[d0:d0 + nn] = r[s:s + nn] * 2   # parity always 0
                        sv[d0:d0 + nn] = v[s:s + nn]
                        s += nn
                        continue
                    # paired: nn edges = 2*npair; first npair are parity 0
                    npair = nn // 2
                    mseq = np.arange(npair)
                    pos0 = d0 + (mseq // 128) * 256 + (mseq % 128)
                    pos1 = pos0 + 128
                    gi[pos0] = coi[s:s + npair]
                    sv[pos0] = v[s:s + npair]
                    gi[pos1] = coi[s + npair:s + nn]
                    sv[pos1] = v[s + npair:s + nn]
                    sp[(d0 // 2) + mseq] = r[s:s + npair]
                    s += nn
        gidx_w[c] = np.tile(gi.reshape(-1, 16).T, (8, 1))
        sp_w[c] = np.tile(sp.reshape(-1, 16).T, (8, 1))
        su_w[c] = np.tile(su.reshape(-1, 16).T, (8, 1))
        sval_w[c] = sv.reshape(-1, 128).T

    return tuple(calls), gidx_w, sp_w, su_w, sval_w


def _build(calls):
    from concourse import bass, mybir, tile, bacc
    from concourse.masks import make_identity

    f32 = mybir.dt.float32
    i16 = mybir.dt.int16
    T = max(e0 + n for _, e0, n, _t in calls)

    nc = bacc.Bacc("TRN2", target_bir_lowering=False, debug=False,
                   num_swdge_queues=4)

    x_d = nc.dram_tensor("x", [N_NODES, D], f32, kind="ExternalInput")
    xT_d = nc.dram_tensor("xlocT", [D, RPC], f32, kind="ExternalInput")
    w_d = nc.dram_tensor("W", [2 * D, OUT], f32, kind="ExternalInput")
    gidx_d = nc.dram_tensor("gidx", [P, T // 16], i16, kind="ExternalInput")
    sp_d = nc.dram_tensor("spair", [P, T // 2 // 16], i16, kind="ExternalInput")
    su_d = nc.dram_tensor("sunp", [P, T // 16], i16, kind="ExternalInput")
    sval_d = nc.dram_tensor("svals", [P, T // 128], f32, kind="ExternalInput")
    out_d = nc.dram_tensor("out", [RPC, OUT], f32, kind="ExternalOutput")
    h_d = nc.dram_tensor("h_acc", [H_PAD, 2 * D], f32)   # 1KB rows, 2 stripes

    relu = mybir.ActivationFunctionType.Relu

    with tile.TileContext(nc) as tc:
        with tc.tile_pool(name="const", bufs=1) as constp, \
             tc.tile_pool(name="mess", bufs=6) as messp, \
             tc.tile_pool(name="meta", bufs=6) as metap, \
             tc.tile_pool(name="dense", bufs=4) as densep, \
             tc.tile_pool(name="psum", bufs=2, space="PSUM") as psump:

            ident = constp.tile([P, P], f32)
            make_identity(nc, ident[:])
            w1 = constp.tile([D, OUT], f32)
            nc.sync.dma_start(out=w1[:], in_=w_d[:D, :])
            w2 = constp.tile([D, OUT], f32)
            nc.sync.dma_start(out=w2[:], in_=w_d[D:, :])
            # preload the whole xlocT arena (tail's x-side inputs, h-independent)
            xta = constp.tile([P, RPC], f32)
            for c0 in range(0, RPC, 2500):
                nc.scalar.dma_start(out=xta[:, c0:c0 + 2500],
                                    in_=xT_d[:, c0:c0 + 2500])

            # zero the striped h accumulator (12800 rows x 1KB)
            zblk = constp.tile([P, 4, 2 * D], f32)
            nc.vector.memset(zblk[:], 0.0)
            for b in range(H_PAD // 512):
                dst = h_d[b * 512:(b + 1) * 512, :].rearrange(
                    "(a p) d -> p a d", p=P)
                nc.scalar.dma_start(out=dst, in_=zblk[:])

            # ---- SpMM phase: gather -> scale -> scatter-add ----
            h_unp = h_d[:].rearrange("r (s d) -> (r s) d", s=2)
            qrr = 0
            for k, e0, n, typ in calls:
                ns = n // 128
                x_chunk = x_d[k * CHUNK:(k + 1) * CHUNK, :]
                gi = metap.tile([P, 2048 // 16], i16, tag="gi")
                nc.sync.dma_start(
                    out=gi[:, :n // 16],
                    in_=gidx_d[:, e0 // 16:(e0 + n) // 16])
                mv = messp.tile([P, 2048 // 128, D], f32, tag="mess")
                for sub in range(0, n, 1024):
                    gn = min(1024, n - sub)
                    nc.gpsimd.dma_gather(
                        mv[:, sub // 128:(sub + gn) // 128, :], x_chunk,
                        gi[:, sub // 16:(sub + gn) // 16], gn, gn, D,
                        queue_num=1 + (qrr % 3))
                    qrr += 1
                sv = metap.tile([P, 2048 // 128], f32, tag="sv")
                nc.sync.dma_start(
                    out=sv[:, :ns],
                    in_=sval_d[:, e0 // 128:(e0 + n) // 128])
                nc.vector.tensor_tensor(
                    out=mv[:, :ns, :], in0=mv[:, :ns, :],
                    in1=sv[:, :ns, None].to_broadcast([P, ns, D]),
                    op=mybir.AluOpType.mult)
                if typ == 0:
                    npr = n // 2
                    si = metap.tile([P, 2048 // 2 // 16], i16, tag="sip")
                    nc.scalar.dma_start(
                        out=si[:, :npr // 16],
                        in_=sp_d[:, e0 // 2 // 16:(e0 // 2 + npr) // 16])
                    nc.gpsimd.dma_scatter_add(
                        h_d[:],
                        mv[:, :ns, :].rearrange("p a d -> p (a d)").rearrange(
                            "p (a d) -> p a d", d=2 * D),
                        si[:, :npr // 16], npr, npr, 2 * D)
                else:
                    si = metap.tile([P, TILE_E // 16], i16, tag="si")
                    nc.scalar.dma_start(
                        out=si[:, :n // 16],
                        in_=su_d[:, e0 // 16:(e0 + n) // 16])
                    nc.gpsimd.dma_scatter_add(
                        h_unp, mv[:, :ns, :], si[:, :n // 16], n, n, D)

            # ---- dense head: out = relu(h @ W1 + x @ W2) ----
            # batched: 4 row-groups (512 rows) per h-load / out-store DMA
            for b in range((RPC + 511) // 512):
                r0 = b * 512
                rb = min(512, RPC - r0)
                nsub = (rb + P - 1) // P
                hl4 = densep.tile([P, 4, 2 * D], f32, tag="hl")
                nc.sync.dma_start(
                    out=hl4[:, :nsub, :],
                    in_=h_d[r0:r0 + nsub * P, :].rearrange(
                        "(a p) d -> p a d", p=P))
                ob4 = densep.tile([P, 4, OUT], f32, tag="ob")
                for a in range(nsub):
                    g0 = r0 + a * P
                    rsz = min(P, RPC - g0)
                    hb = densep.tile([P, D], f32, tag="hb")
                    nc.vector.tensor_add(out=hb[:rsz, :],
                                         in0=hl4[:rsz, a, :D],
                                         in1=hl4[:rsz, a, D:])
                    pt = psump.tile([P, P], f32, tag="pt")
                    nc.tensor.transpose(pt[:, :rsz], hb[:rsz, :],
                                        ident[:rsz, :rsz])
                    hT = densep.tile([P, P], f32, tag="hT")
                    nc.vector.tensor_copy(hT[:, :rsz], pt[:, :rsz])
                    po = psump.tile([P, OUT], f32, tag="po")
                    nc.tensor.matmul(po[:rsz, :], hT[:, :rsz], w1[:],
                                     start=True, stop=False)
                    nc.tensor.matmul(po[:rsz, :], xta[:, g0:g0 + rsz], w2[:],
                                     start=False, stop=True)
                    nc.scalar.activation(ob4[:rsz, a, :], po[:rsz, :], relu)
                if rb == 512:
                    nc.scalar.dma_start(
                        out=out_d[r0:r0 + 512, :].rearrange(
                            "(a p) d -> p a d", p=P),
                        in_=ob4[:])
                else:
                    for a in range(nsub):
                        g0 = r0 + a * P
                        rsz = min(P, RPC - g0)
                        nc.scalar.dma_start(out=out_d[g0:g0 + rsz, :],
                                            in_=ob4[:rsz, a, :])

    nc.compile()
    return nc


def _get_nc(calls):
    nc = _compiled.get(calls)
    if nc is None:
        nc = _build(calls)
        _compiled[calls] = nc
    return nc


def _make_in_maps(x, W, calls, gidx_w, sp_w, su_w, sval_w):
    x = np.ascontiguousarray(np.asarray(x, np.float32))
    W = np.ascontiguousarray(np.asarray(W, np.float32))
    in_maps = []
    for c in range(NCORES):
        xloc = x[c * RPC:(c + 1) * RPC]
        in_maps.append({
            "x": x,
            "xlocT": np.ascontiguousarray(xloc.T),
            "W": W,
            "gidx": gidx_w[c],
            "spair": sp_w[c],
            "sunp": su_w[c],
            "svals": sval_w[c],
        })
    return in_maps


def _install_trace_shims():
    """Make trace=True work in this container: provide antenv.axon_hooks
    (ctypes NTFF profiling via the axon PJRT .so) and stub the artifact
    upload (no bucket access here)."""
    import contextlib
    import ctypes
    import types

    try:
        import antenv.axon_hooks  # noqa: F401
        has_hooks = True
    except ImportError:
        has_hooks = False
    if not has_hooks:
        so_path = "/opt/axon/libaxon_pjrt.so"
        lib = ctypes.CDLL(so_path)
        if hasattr(lib, "axon_start_nrt_profile"):
            lib.axon_start_nrt_profile.argtypes = [
                ctypes.POINTER(ctypes.c_int64), ctypes.c_size_t]
            lib.axon_start_nrt_profile.restype = ctypes.c_int64
            lib.axon_stop_nrt_profile.argtypes = [ctypes.c_char_p]
            lib.axon_stop_nrt_profile.restype = ctypes.c_int64

            @contextlib.contextmanager
            def _hook(output_dir, device_ids):
                import jax
                jax.devices()
                if device_ids:
                    ids = (ctypes.c_int64 * len(device_ids))(*device_ids)
                    rc = lib.axon_start_nrt_profile(ids, len(device_ids))
                else:
                    rc = lib.axon_start_nrt_profile(None, 0)
                if rc != 0:
                    raise RuntimeError(f"axon_start_nrt_profile rc={rc}")
                try:
                    yield
                finally:
                    n = lib.axon_stop_nrt_profile(str(output_dir).encode())
                    if n <= 0:
                        print(f"ntff profile: rc={n} (no files?) at {output_dir}")

            mod = types.ModuleType("antenv.axon_hooks")
            mod.get_axon_ntff_profile_hook = lambda: _hook
            mod.set_axon_ntff_profile_hook = lambda h: None
            sys.modules["antenv.axon_hooks"] = mod

    import concourse.bass_utils as bu
    bu.upload_artifacts = lambda tmpdir: f"local:{tmpdir}"


def _run(x, adj_rows, adj_cols, adj_vals, W, trace=False):
    from concourse.bass_utils import run_bass_kernel_spmd
    if trace:
        try:
            _install_trace_shims()
        except Exception as e:  # tracing is best-effort
            print("trace shim install failed:", e)
    calls, gidx_w, sp_w, su_w, sval_w = _prep(adj_rows, adj_cols, adj_vals)
    nc = _get_nc(calls)
    in_maps = _make_in_maps(x, W, calls, gidx_w, sp_w, su_w, sval_w)
    res = run_bass_kernel_spmd(nc, in_maps, list(range(NCORES)), trace=trace)
    out = np.concatenate([res.results[c]["out"] for c in range(NCORES)], axis=0)
    return out, res


def kernel(x, adj_rows, adj_cols, adj_vals, W):
    out, _ = _run(x, adj_rows, adj_cols, adj_vals, W, trace=False)
    return out



# revision 2
# speedup vs baseline: 2.4922x; 2.4922x over previous
"""Distributed Trainium2 kernel for GNN message passing (COO SpMM + dense head).

out = relu((A @ x) @ W[:128] + x @ W[128:])   with A given as COO (rows, cols, vals)

Strategy (8 NeuronCores, SPMD single graph):
  - Rows (destinations) sharded across cores: core c owns rows [c*12500, (c+1)*12500).
  - x replicated to every core's DRAM (bf16) via its input map; no collectives.
  - SpMM = hardware gather + SEGMENT-SUM VIA TENSOR-ENGINE MATMULS (no SWDGE
    scatter at all -- scatter-add descriptor generation was the baseline's
    dominant Pool-engine cost at ~6ns/descriptor):
      * edges sorted by (row-group g = r//128, col-chunk k, row); per (g,k)
        cell the edge count is padded to a shared capacity across cores
        (SPMD: one program). Pad slots hold gather idx -1 (trailing negatives
        are SKIPPED by the Q7 descriptor generator -> free) and all-zero
        columns in S.
      * per 128-edge subtile: one bf16 matmul  psum_g += msgs^T @ S_sub where
        S_sub[i, j] = val_i * onehot(r_i - 128g == j) is HOST-precomputed bf16
        (values folded in -> no vector work in the inner loop).
      * psum_g accumulates the whole group's 4 chunk-cells, then one scalar
        activation copies it (cast bf16) into an SBUF-resident hT arena
        [128 feat x 12800 rows]. h never touches DRAM.
  - Dense head overlapped with SpMM: every 4 groups, outT = relu(W1^T @ hT +
    W2^T @ xT) with N=512 matmuls (W stationary), relu on ScalarE, contiguous
    store of outT [128 x 12800]; host transposes at the end.
"""

import sys

if "/opt/trn_rl_repo" not in sys.path:
    sys.path.insert(0, "/opt/trn_rl_repo")

import numpy as np
import ml_dtypes

BF16 = ml_dtypes.bfloat16

N_NODES = 100000
N_EDGES = 600000
D = 128
OUT = 128
P = 128
NCORES = 8
RPC = N_NODES // NCORES          # 12500 rows per core
NCHUNK = 4
CHUNK = N_NODES // NCHUNK        # 25000 (< 32768 so int16 gather idx works)
NG = (RPC + P - 1) // P          # 98 row-groups of 128 rows
RPAD = 12800                     # 25 head batches x 512 rows
NB = RPAD // 512                 # 25 head batches

_compiled = {}


def _prep(adj_rows, adj_cols, adj_vals):
    """Per-core uniform-shape gather idx + segment-matrix streams.

    Edges of core c sorted by (g=r//128, chunk=col//25000, r). Cell (g,k)
    capacity = max over cores, rounded up to 128 (subtile size). Streams:
      gi : int16 gather indices (col % 25000), -1 pad (trailing per cell)
      S  : bf16 [128, T]; edge at stream pos i -> S[i%128,
           (i//128)*128 + (r - 128g)] = val. Pad slots: zero columns.
    """
    rows = np.asarray(adj_rows).astype(np.int64)
    cols = np.asarray(adj_cols).astype(np.int64)
    vals = np.asarray(adj_vals).astype(np.float32)

    per_core = []
    counts = np.zeros((NCORES, NG * NCHUNK), np.int64)
    for c in range(NCORES):
        m = (rows >= c * RPC) & (rows < (c + 1) * RPC)
        r = rows[m] - c * RPC
        co = cols[m]
        v = vals[m]
        g = r >> 7
        ch = co // CHUNK
        o = np.lexsort((r, ch, g))
        r, co, v, ch, g = r[o], co[o], v[o], ch[o], g[o]
        cell = g * NCHUNK + ch
        counts[c] = np.bincount(cell, minlength=NG * NCHUNK)
        per_core.append((r, co, v, cell))

    caps = ((counts.max(axis=0) + 127) // 128) * 128
    caps = np.maximum(caps, 128)                      # [NG*NCHUNK]
    off = np.concatenate([[0], np.cumsum(caps)]).astype(np.int64)
    T = int(off[-1])

    gi_w = np.zeros((NCORES, P, T // 16), np.int16)
    s_w = np.zeros((NCORES, P, T), BF16)
    for c in range(NCORES):
        r, co, v, cell = per_core[c]
        n = len(r)
        starts = np.concatenate([[0], np.cumsum(counts[c])])
        pos = off[cell] + (np.arange(n) - starts[cell])
        gi = np.full(T, -1, np.int16)
        gi[pos] = (co % CHUNK).astype(np.int16)
        gi_w[c] = np.tile(gi.reshape(-1, 16).T, (8, 1))
        S = np.zeros((P, T), np.float32)
        S[pos % P, (pos // P) * P + (r - ((r >> 7) << 7))] = v
        s_w[c] = S.astype(BF16)

    return tuple(int(x) for x in caps), gi_w, s_w


def _build(caps):
    from concourse import bass, mybir, tile, bacc

    f32 = mybir.dt.float32
    bf16 = mybir.dt.bfloat16
    i16 = mybir.dt.int16
    relu = mybir.ActivationFunctionType.Relu
    copyf = mybir.ActivationFunctionType.Copy

    caps = np.asarray(caps, np.int64)                 # [NG*NCHUNK]
    off = np.concatenate([[0], np.cumsum(caps)]).astype(np.int64)
    T = int(off[-1])
    maxsub = int(caps.max()) // P

    # S mega-tile bounds: one per 8 groups
    nmega = (NG + 7) // 8
    mega_lo = [int(off[m * 8 * NCHUNK]) for m in range(nmega)]
    mega_hi = [int(off[min((m + 1) * 8, NG) * NCHUNK]) for m in range(nmega)]
    mega_max = max(hi - lo for lo, hi in zip(mega_lo, mega_hi))

    nc = bacc.Bacc("TRN2", target_bir_lowering=False, debug=False,
                   num_swdge_queues=4)

    x_d = nc.dram_tensor("x", [N_NODES, D], bf16, kind="ExternalInput")
    xT_d = nc.dram_tensor("xlocT", [D, RPAD], bf16, kind="ExternalInput")
    w_d = nc.dram_tensor("W", [2 * D, OUT], bf16, kind="ExternalInput")
    gi_d = nc.dram_tensor("gidx", [P, T // 16], i16, kind="ExternalInput")
    s_d = nc.dram_tensor("smat", [P, T], bf16, kind="ExternalInput")
    outT_d = nc.dram_tensor("outT", [OUT, RPAD], f32, kind="ExternalOutput")

    with tile.TileContext(nc) as tc:
        with tc.tile_pool(name="const", bufs=1) as constp, \
             tc.tile_pool(name="smega", bufs=2) as smp, \
             tc.tile_pool(name="mess", bufs=8) as mvp, \
             tc.tile_pool(name="outb", bufs=2) as outp, \
             tc.tile_pool(name="psseg", bufs=3, space="PSUM") as segp, \
             tc.tile_pool(name="pshead", bufs=2, space="PSUM") as headp:

            # gather idx arena (whole stream resident)
            gi_t = constp.tile([P, T // 16], i16)
            qn = T // 16 // 4
            for q in range(4):
                q0 = q * qn
                q1 = (q + 1) * qn if q < 3 else T // 16
                nc.sync.dma_start(out=gi_t[:, q0:q1], in_=gi_d[:, q0:q1])

            w1 = constp.tile([D, OUT], bf16)
            nc.scalar.dma_start(out=w1[:], in_=w_d[:D, :])
            w2 = constp.tile([D, OUT], bf16)
            nc.scalar.dma_start(out=w2[:], in_=w_d[D:, :])
            xta = constp.tile([P, RPAD], bf16)
            for q in range(4):
                nc.scalar.dma_start(out=xta[:, q * 3200:(q + 1) * 3200],
                                    in_=xT_d[:, q * 3200:(q + 1) * 3200])

            arena = constp.tile([P, RPAD], bf16)
            # groups only cover [0, NG*128); memset the tail once
            nc.vector.memset(arena[:, NG * P:], 0.0)

            # pre-zero the gather buffers: trailing-negative pad slots are
            # skipped by descgen and would otherwise read uninitialized SBUF
            # (NaN * 0 = NaN in the matmul)
            mvz = []
            for b in range(8):
                mv = mvp.tile([P, maxsub, D], bf16, tag="mv")
                nc.vector.memset(mv[:], 0.0)
                mvz.append(mv)
            del mvz

            qrr = 0
            for g in range(NG):
                m = g // 8
                if g % 8 == 0:
                    sm = smp.tile([P, mega_max], bf16, tag="sm")
                    msz = mega_hi[m] - mega_lo[m]
                    nc.sync.dma_start(out=sm[:, :msz],
                                      in_=s_d[:, mega_lo[m]:mega_hi[m]])
                    cur_sm, cur_lo = sm, mega_lo[m]

                psg = segp.tile([P, P], f32, tag="ps")
                nsub_tot = sum(int(caps[g * NCHUNK + k]) for k in range(NCHUNK)) // P
                sdone = 0
                for k in range(NCHUNK):
                    cap = int(caps[g * NCHUNK + k])
                    base = int(off[g * NCHUNK + k])
                    ns = cap // P
                    mv = mvp.tile([P, maxsub, D], bf16, tag="mv")
                    nc.gpsimd.dma_gather(
                        mv[:, :ns, :], x_d[k * CHUNK:(k + 1) * CHUNK, :],
                        gi_t[:, base // 16:(base + cap) // 16], cap, cap, D,
                        queue_num=qrr % 4)
                    qrr += 1
                    for s in range(ns):
                        so = base + s * P - cur_lo
                        nc.tensor.matmul(psg[:], mv[:, s, :],
                                         cur_sm[:, so:so + P],
                                         start=(sdone == 0),
                                         stop=(sdone == nsub_tot - 1))
                        sdone += 1
                nc.scalar.activation(arena[:, g * P:(g + 1) * P], psg[:], copyf)

                if g % 4 == 3:
                    c0 = (g // 4) * 512
                    ph = headp.tile([P, 512], f32, tag="ph")
                    nc.tensor.matmul(ph[:], w1[:], arena[:, c0:c0 + 512],
                                     start=True, stop=False)
                    nc.tensor.matmul(ph[:], w2[:], xta[:, c0:c0 + 512],
                                     start=False, stop=True)
                    ob = outp.tile([P, 512], f32, tag="ob")
                    nc.scalar.activation(ob[:], ph[:], relu)
                    nc.scalar.dma_start(out=outT_d[:, c0:c0 + 512], in_=ob[:])

            # final head batch: groups 96..97 plus zero tail (12288..12800)
            c0 = (NG // 4) * 512
            assert c0 == 12288
            ph = headp.tile([P, 512], f32, tag="ph")
            nc.tensor.matmul(ph[:], w1[:], arena[:, c0:c0 + 512],
                             start=True, stop=False)
            nc.tensor.matmul(ph[:], w2[:], xta[:, c0:c0 + 512],
                             start=False, stop=True)
            ob = outp.tile([P, 512], f32, tag="ob")
            nc.scalar.activation(ob[:], ph[:], relu)
            nc.scalar.dma_start(out=outT_d[:, c0:c0 + 512], in_=ob[:])

    nc.compile()
    return nc


def _get_nc(caps):
    nc = _compiled.get(caps)
    if nc is None:
        nc = _build(caps)
        _compiled[caps] = nc
    return nc


def _make_in_maps(x, W, gi_w, s_w):
    x = np.asarray(x, np.float32)
    xb = x.astype(BF16)
    Wb = np.ascontiguousarray(np.asarray(W, np.float32).astype(BF16))
    in_maps = []
    for c in range(NCORES):
        xloc = np.zeros((D, RPAD), BF16)
        xloc[:, :RPC] = xb[c * RPC:(c + 1) * RPC].T
        in_maps.append({
            "x": xb,
            "xlocT": np.ascontiguousarray(xloc),
            "W": Wb,
            "gidx": gi_w[c],
            "smat": s_w[c],
        })
    return in_maps


def _install_trace_shims():
    """Make trace=True work in this container: provide antenv.axon_hooks
    (ctypes NTFF profiling via the axon PJRT .so) and stub the artifact
    upload (no bucket access here)."""
    import contextlib
    import ctypes
    import types

    try:
        import antenv.axon_hooks  # noqa: F401
        has_hooks = True
    except ImportError:
        has_hooks = False
    if not has_hooks:
        so_path = "/opt/axon/libaxon_pjrt.so"
        lib = ctypes.CDLL(so_path)
        if hasattr(lib, "axon_start_nrt_profile"):
            lib.axon_start_nrt_profile.argtypes = [
                ctypes.POINTER(ctypes.c_int64), ctypes.c_size_t]
            lib.axon_start_nrt_profile.restype = ctypes.c_int64
            lib.axon_stop_nrt_profile.argtypes = [ctypes.c_char_p]
            lib.axon_stop_nrt_profile.restype = ctypes.c_int64

            @contextlib.contextmanager
            def _hook(output_dir, device_ids):
                import jax
                jax.devices()
                if device_ids:
                    ids = (ctypes.c_int64 * len(device_ids))(*device_ids)
                    rc = lib.axon_start_nrt_profile(ids, len(device_ids))
                else:
                    rc = lib.axon_start_nrt_profile(None, 0)
                if rc != 0:
                    raise RuntimeError(f"axon_start_nrt_profile rc={rc}")
                try:
                    yield
                finally:
                    n = lib.axon_stop_nrt_profile(str(output_dir).encode())
                    if n <= 0:
                        print(f"ntff profile: rc={n} (no files?) at {output_dir}")

            mod = types.ModuleType("antenv.axon_hooks")
            mod.get_axon_ntff_profile_hook = lambda: _hook
            mod.set_axon_ntff_profile_hook = lambda h: None
            sys.modules["antenv.axon_hooks"] = mod

    import concourse.bass_utils as bu
    bu.upload_artifacts = lambda tmpdir: f"local:{tmpdir}"


def _run(x, adj_rows, adj_cols, adj_vals, W, trace=False):
    from concourse.bass_utils import run_bass_kernel_spmd
    if trace:
        try:
            _install_trace_shims()
        except Exception as e:  # tracing is best-effort
            print("trace shim install failed:", e)
    caps, gi_w, s_w = _prep(adj_rows, adj_cols, adj_vals)
    nc = _get_nc(caps)
    in_maps = _make_in_maps(x, W, gi_w, s_w)
    res = run_bass_kernel_spmd(nc, in_maps, list(range(NCORES)), trace=trace)
    out = np.concatenate(
        [np.asarray(res.results[c]["outT"])[:, :RPC].T for c in range(NCORES)],
        axis=0)
    return np.ascontiguousarray(out, dtype=np.float32), res


def kernel(x, adj_rows, adj_cols, adj_vals, W):
    out, _ = _run(x, adj_rows, adj_cols, adj_vals, W, trace=False)
    return out


# revision 5
# speedup vs baseline: 3.1939x; 1.2816x over previous
"""Distributed Trainium2 kernel for GNN message passing (COO SpMM + dense head).

out = relu((A @ x) @ W[:128] + x @ W[128:])   with A given as COO (rows, cols, vals)

Strategy (8 NeuronCores, SPMD single graph):
  - Rows (destinations) sharded across cores: core c owns rows [c*12500, (c+1)*12500).
  - x replicated to every core's DRAM (bf16) via its input map; no collectives.
  - SpMM = hardware gather + SEGMENT-SUM VIA TENSOR-ENGINE MATMULS (no SWDGE
    scatter at all -- scatter-add descriptor generation was the baseline's
    dominant Pool-engine cost at ~6ns/descriptor):
      * edges sorted by (col-chunk k, row-group g = r//128, row); per (k,g)
        cell the edge count is padded to a shared 128-aligned capacity across
        cores (SPMD: one program). Pad slots hold gather idx -1 (skipped when
        trailing in a call) or 0 (mid-call; S column is zero either way).
      * gather calls are 1024-slot windows of each chunk's stream -- per-call
        fixed overhead on the Q7 descgen cores is ~600ns, so few big calls
        (98) beat per-cell calls (392).
      * per 128-edge subtile: one bf16 matmul  psum_g += msgs^T @ S_sub where
        S_sub[i, j] = val_i * onehot(r_i - 128g == j) is HOST-precomputed bf16
        (values folded in -> no vector work in the inner loop). psum_g
        accumulates the whole group's 4 chunk-cells (cells located inside the
        big gather windows by static offset arithmetic), then one scalar
        activation copies it (cast bf16) into an SBUF-resident hT arena
        [128 feat x 12800 rows]. h never touches DRAM.
  - Dense head overlapped with SpMM: every 4 groups, outT = relu(W1^T @ hT +
    W2^T @ xT) with N=512 matmuls (W stationary), relu on ScalarE, contiguous
    store of outT [128 x 12800]; host transposes at the end.
"""

import sys

if "/opt/trn_rl_repo" not in sys.path:
    sys.path.insert(0, "/opt/trn_rl_repo")

import numpy as np
import ml_dtypes

BF16 = ml_dtypes.bfloat16

N_NODES = 100000
N_EDGES = 600000
D = 128
OUT = 128
P = 128
NCORES = 8
RPC = N_NODES // NCORES          # 12500 rows per core
NCHUNK = 4
CHUNK = N_NODES // NCHUNK        # 25000 (< 32768 so int16 gather idx works)
NG = (RPC + P - 1) // P          # 98 row-groups of 128 rows
RPAD = 12800                     # 25 head batches x 512 rows
CALL = 1024                      # gather slots per SWDGE call (ring limit)

_compiled = {}


def _prep(adj_rows, adj_cols, adj_vals):
    """Per-core uniform-shape gather idx + segment-matrix streams.

    Edges of core c sorted by (chunk=col//25000, r). Cell (k,g) capacity =
    max over cores, rounded up to 128 (subtile size). Four chunk-major
    streams concatenated; stream k starts at GO[k] (1024-aligned). Streams:
      gi : int16 gather indices (col % 25000); pads are 0 (mid-call) or -1
           (trailing in the last call of a stream)
      S  : bf16 [128, T]; edge at global slot i -> S[i%128,
           (i//128)*128 + (r - 128g)] = val. Pad slots: zero columns.
    """
    rows = np.asarray(adj_rows).astype(np.int64)
    cols = np.asarray(adj_cols).astype(np.int64)
    vals = np.asarray(adj_vals).astype(np.float32)

    per_core = []
    counts = np.zeros((NCORES, NCHUNK * NG), np.int64)
    for c in range(NCORES):
        m = (rows >= c * RPC) & (rows < (c + 1) * RPC)
        r = rows[m] - c * RPC
        co = cols[m]
        v = vals[m]
        ch = co // CHUNK
        o = np.lexsort((r, ch))
        r, co, v, ch = r[o], co[o], v[o], ch[o]
        cell = ch * NG + (r >> 7)
        counts[c] = np.bincount(cell, minlength=NCHUNK * NG)
        per_core.append((r, co, v, cell))

    caps = ((counts.max(axis=0) + 127) // 128) * 128
    caps = np.maximum(caps, 128).reshape(NCHUNK, NG)    # [k, g]
    sk = caps.sum(axis=1)                               # stream k size
    go = np.concatenate([[0], np.cumsum(((sk + CALL - 1) // CALL) * CALL)])
    T = int(go[-1])
    # stream-local cell offsets
    cell_off = np.zeros((NCHUNK, NG), np.int64)
    cell_off[:, 1:] = np.cumsum(caps, axis=1)[:, :-1]
    # global slot offset per cell, flattened in (k, g) order
    gcell_off = (cell_off + go[:-1, None]).reshape(-1)

    gi_w = np.zeros((NCORES, P, T // 16), np.int16)
    s_w = np.zeros((NCORES, P, T), BF16)
    for c in range(NCORES):
        r, co, v, cell = per_core[c]
        n = len(r)
        starts = np.concatenate([[0], np.cumsum(counts[c])])
        pos = gcell_off[cell] + (np.arange(n) - starts[cell])
        gi = np.zeros(T, np.int16)
        for k in range(NCHUNK):                 # trailing pads of each stream
            gi[int(go[k]) + int(sk[k]):int(go[k + 1])] = -1
        gi[pos] = (co % CHUNK).astype(np.int16)
        gi_w[c] = np.tile(gi.reshape(-1, 16).T, (8, 1))
        S = np.zeros((P, T), np.float32)
        S[pos % P, (pos // P) * P + (r - ((r >> 7) << 7))] = v
        s_w[c] = S.astype(BF16)

    key = tuple(int(x) for x in caps.reshape(-1))
    return key, gi_w, s_w


def _build(key):
    from concourse import bass, mybir, tile, bacc

    f32 = mybir.dt.float32
    bf16 = mybir.dt.bfloat16
    i16 = mybir.dt.int16
    relu = mybir.ActivationFunctionType.Relu
    copyf = mybir.ActivationFunctionType.Copy

    caps = np.asarray(key, np.int64).reshape(NCHUNK, NG)
    sk = caps.sum(axis=1)
    go = np.concatenate([[0], np.cumsum(((sk + CALL - 1) // CALL) * CALL)])
    T = int(go[-1])
    cell_off = np.zeros((NCHUNK, NG), np.int64)
    cell_off[:, 1:] = np.cumsum(caps, axis=1)[:, :-1]

    # S mega-tile windows: per (4-group block m, stream k)
    MBLK = 4
    nmega = (NG + MBLK - 1) // MBLK
    def mwin(m, k):
        ge = min((m + 1) * MBLK, NG) - 1
        lo = int(cell_off[k, m * MBLK])
        hi = int(cell_off[k, ge] + caps[k, ge])
        return lo, hi
    mega_max = max(mwin(m, k)[1] - mwin(m, k)[0]
                   for m in range(nmega) for k in range(NCHUNK))

    nc = bacc.Bacc("TRN2", target_bir_lowering=False, debug=False,
                   num_swdge_queues=4)

    x_d = nc.dram_tensor("x", [N_NODES, D], bf16, kind="ExternalInput")
    xT_d = nc.dram_tensor("xlocT", [D, RPAD], bf16, kind="ExternalInput")
    w_d = nc.dram_tensor("W", [2 * D, OUT], bf16, kind="ExternalInput")
    gi_d = nc.dram_tensor("gidx", [P, T // 16], i16, kind="ExternalInput")
    s_d = nc.dram_tensor("smat", [P, T], bf16, kind="ExternalInput")
    outT_d = nc.dram_tensor("outT", [OUT, RPAD], f32, kind="ExternalOutput")

    with tile.TileContext(nc) as tc:
        with tc.tile_pool(name="const", bufs=1) as constp, \
             tc.tile_pool(name="smega", bufs=8) as smp, \
             tc.tile_pool(name="mess", bufs=10) as mvp, \
             tc.tile_pool(name="outb", bufs=2) as outp, \
             tc.tile_pool(name="psseg", bufs=3, space="PSUM") as segp, \
             tc.tile_pool(name="pshead", bufs=2, space="PSUM") as headp:

            # gather idx arena (whole stream resident)
            gi_t = constp.tile([P, T // 16], i16)
            qn = T // 16 // 4
            for q in range(4):
                q0 = q * qn
                q1 = (q + 1) * qn if q < 3 else T // 16
                nc.sync.dma_start(out=gi_t[:, q0:q1], in_=gi_d[:, q0:q1])

            w1 = constp.tile([D, OUT], bf16)
            nc.scalar.dma_start(out=w1[:], in_=w_d[:D, :])
            w2 = constp.tile([D, OUT], bf16)
            nc.scalar.dma_start(out=w2[:], in_=w_d[D:, :])
            xta = constp.tile([P, RPAD], bf16)
            for q in range(4):
                nc.scalar.dma_start(out=xta[:, q * 3200:(q + 1) * 3200],
                                    in_=xT_d[:, q * 3200:(q + 1) * 3200])

            arena = constp.tile([P, RPAD], bf16)
            # groups only cover [0, NG*128); memset the tail once
            nc.vector.memset(arena[:, NG * P:], 0.0)

            # pre-zero the gather buffers: trailing-negative pad slots are
            # skipped by descgen and would otherwise read uninitialized SBUF
            # (NaN * 0 = NaN in the matmul)
            for b in range(10):
                mv = mvp.tile([P, CALL // P, D], bf16, tag="mv")
                nc.vector.memset(mv[:], 0.0)

            ncalls = [(int(sk[k]) + CALL - 1) // CALL for k in range(NCHUNK)]
            callidx = [0, 0, 0, 0]
            mvtiles = [dict() for _ in range(NCHUNK)]
            smtiles = [None] * NCHUNK
            smlo = [0] * NCHUNK
            qrr = 0

            for g in range(NG):
                m = g // MBLK
                if g % MBLK == 0:
                    for k in range(NCHUNK):
                        lo, hi = mwin(m, k)
                        sm = smp.tile([P, mega_max], bf16, tag=f"sm{k}")
                        nc.sync.dma_start(
                            out=sm[:, :hi - lo],
                            in_=s_d[:, int(go[k]) + lo:int(go[k]) + hi])
                        smtiles[k], smlo[k] = sm, lo

                # issue gather calls covering this group's cells
                for k in range(NCHUNK):
                    need = int(cell_off[k, g] + caps[k, g])
                    while callidx[k] * CALL < need:
                        w = callidx[k]
                        nn = min(CALL, int(sk[k]) - w * CALL)
                        nn = ((nn + 127) // 128) * 128
                        mv = mvp.tile([P, CALL // P, D], bf16, tag="mv")
                        b0 = int(go[k]) + w * CALL
                        nc.gpsimd.dma_gather(
                            mv[:, :nn // P, :],
                            x_d[k * CHUNK:(k + 1) * CHUNK, :],
                            gi_t[:, b0 // 16:(b0 + nn) // 16], nn, nn, D,
                            queue_num=1 + qrr % 3)
                        qrr += 1
                        mvtiles[k][w] = mv
                        if w >= 3:
                            mvtiles[k].pop(w - 3, None)
                        callidx[k] += 1

                psg = segp.tile([P, P], f32, tag="ps")
                nsub_tot = int(caps[:, g].sum()) // P
                sdone = 0
                for k in range(NCHUNK):
                    for s in range(int(caps[k, g]) // P):
                        q = int(cell_off[k, g]) + s * P
                        mv = mvtiles[k][q // CALL]
                        nc.tensor.matmul(
                            psg[:], mv[:, (q % CALL) // P, :],
                            smtiles[k][:, q - smlo[k]:q - smlo[k] + P],
                            start=(sdone == 0), stop=(sdone == nsub_tot - 1))
                        sdone += 1
                nc.scalar.activation(arena[:, g * P:(g + 1) * P], psg[:], copyf)

                if g % 4 == 3:
                    c0 = (g // 4) * 512
                    ph = headp.tile([P, 512], f32, tag="ph")
                    nc.tensor.matmul(ph[:], w1[:], arena[:, c0:c0 + 512],
                                     start=True, stop=False)
                    nc.tensor.matmul(ph[:], w2[:], xta[:, c0:c0 + 512],
                                     start=False, stop=True)
                    ob = outp.tile([P, 512], f32, tag="ob")
                    nc.scalar.activation(ob[:], ph[:], relu)
                    nc.scalar.dma_start(out=outT_d[:, c0:c0 + 512], in_=ob[:])

            # final head batch: groups 96..97 plus zero tail (12288..12800)
            c0 = (NG // 4) * 512
            assert c0 == 12288
            ph = headp.tile([P, 512], f32, tag="ph")
            nc.tensor.matmul(ph[:], w1[:], arena[:, c0:c0 + 512],
                             start=True, stop=False)
            nc.tensor.matmul(ph[:], w2[:], xta[:, c0:c0 + 512],
                             start=False, stop=True)
            ob = outp.tile([P, 512], f32, tag="ob")
            nc.scalar.activation(ob[:], ph[:], relu)
            nc.scalar.dma_start(out=outT_d[:, c0:c0 + 512], in_=ob[:])

    nc.compile()
    return nc


def _get_nc(key):
    nc = _compiled.get(key)
    if nc is None:
        nc = _build(key)
        _compiled[key] = nc
    return nc


def _make_in_maps(x, W, gi_w, s_w):
    x = np.asarray(x, np.float32)
    xb = x.astype(BF16)
    Wb = np.ascontiguousarray(np.asarray(W, np.float32).astype(BF16))
    in_maps = []
    for c in range(NCORES):
        xloc = np.zeros((D, RPAD), BF16)
        xloc[:, :RPC] = xb[c * RPC:(c + 1) * RPC].T
        in_maps.append({
            "x": xb,
            "xlocT": np.ascontiguousarray(xloc),
            "W": Wb,
            "gidx": gi_w[c],
            "smat": s_w[c],
        })
    return in_maps


def _install_trace_shims():
    """Make trace=True work in this container: provide antenv.axon_hooks
    (ctypes NTFF profiling via the axon PJRT .so) and stub the artifact
    upload (no bucket access here)."""
    import contextlib
    import ctypes
    import types

    try:
        import antenv.axon_hooks  # noqa: F401
        has_hooks = True
    except ImportError:
        has_hooks = False
    if not has_hooks:
        so_path = "/opt/axon/libaxon_pjrt.so"
        lib = ctypes.CDLL(so_path)
        if hasattr(lib, "axon_start_nrt_profile"):
            lib.axon_start_nrt_profile.argtypes = [
                ctypes.POINTER(ctypes.c_int64), ctypes.c_size_t]
            lib.axon_start_nrt_profile.restype = ctypes.c_int64
            lib.axon_stop_nrt_profile.argtypes = [ctypes.c_char_p]
            lib.axon_stop_nrt_profile.restype = ctypes.c_int64

            @contextlib.contextmanager
            def _hook(output_dir, device_ids):
                import jax
                jax.devices()
                if device_ids:
                    ids = (ctypes.c_int64 * len(device_ids))(*device_ids)
                    rc = lib.axon_start_nrt_profile(ids, len(device_ids))
                else:
                    rc = lib.axon_start_nrt_profile(None, 0)
                if rc != 0:
                    raise RuntimeError(f"axon_start_nrt_profile rc={rc}")
                try:
                    yield
                finally:
                    n = lib.axon_stop_nrt_profile(str(output_dir).encode())
                    if n <= 0:
                        print(f"ntff profile: rc={n} (no files?) at {output_dir}")

            mod = types.ModuleType("antenv.axon_hooks")
            mod.get_axon_ntff_profile_hook = lambda: _hook
            mod.set_axon_ntff_profile_hook = lambda h: None
            sys.modules["antenv.axon_hooks"] = mod

    import concourse.bass_utils as bu
    bu.upload_artifacts = lambda tmpdir: f"local:{tmpdir}"


def _run(x, adj_rows, adj_cols, adj_vals, W, trace=False):
    from concourse.bass_utils import run_bass_kernel_spmd
    if trace:
        try:
            _install_trace_shims()
        except Exception as e:  # tracing is best-effort
            print("trace shim install failed:", e)
    key, gi_w, s_w = _prep(adj_rows, adj_cols, adj_vals)
    nc = _get_nc(key)
    in_maps = _make_in_maps(x, W, gi_w, s_w)
    res = run_bass_kernel_spmd(nc, in_maps, list(range(NCORES)), trace=trace)
    out = np.concatenate(
        [np.asarray(res.results[c]["outT"])[:, :RPC].T for c in range(NCORES)],
        axis=0)
    return np.ascontiguousarray(out, dtype=np.float32), res


def kernel(x, adj_rows, adj_cols, adj_vals, W):
    out, _ = _run(x, adj_rows, adj_cols, adj_vals, W, trace=False)
    return out
